# revision 19
# baseline (speedup 1.0000x reference)
"""Trainium2 Bass kernel: Tacotron-style location-sensitive attention.

reference math (per batch b):
  q      = hidden[b] @ Wq                              [1, 128]
  loc    = conv1d(aw[b], Wconv, pad=15) + bconv        [32, T]
  pl     = loc.T @ Wloc                                [T, 128]
  e[t]   = v . tanh(q + pl[t] + pm[b,t])               [T]
  p      = softmax(e)                                  [T]
  ctx    = p @ mem[b]                                  [512]

Sharding: pure data parallel, 8 batches per core on 8 cores.

Layout (per core, 8 batches). T is host-padded 2000->2048 so every on-chip
tile uses the full 128 partitions (125-partition DMAs measure 2.3x slower).
t factors as t = 128*g + p (group g in [0,16), partition p in [0,128)):
  - conv + location projection collapse into ONE matmul per 512-col chunk:
      pl^T[d, t] = Wcomb[0:62].T @ shifted[62, t]
    with shifted[31*ci+k, t] = aw[ci, t+k-15] built by an overlapped-window
    SBUF->SBUF DMA, and Wcomb = Wconv_flat.T @ Wloc computed on device once.
    The bconv term (bconv @ Wloc) and the query q fold into the tanh bias
    (d on partitions).
  - pm tiles are PE-transposed and accumulated into the same PSUM tile as
    the pl matmul (pl first with start=True, transposes start=False).
  - e^T columns [128,1] via PE (lhsT = tanh tile, rhs = v), exp on ACT,
    the 48 padded t slots are zeroed in p, context = 16 accumulating
    matmuls streaming mem tiles [128, 512].
  - softmax norm: column sums via PE + ones, replicated total via a
    ones[16,16] matmul, reciprocal on DVE, scale + DMA out on the scalar
    (ACT) HWDGE queue so stores never block the sync-load queue.
"""

import os
import sys

sys.path.insert(0, "/opt/trn_rl_repo")

import numpy as np
from contextlib import ExitStack

import concourse.bass as bass
import concourse.bacc as bacc
import concourse.tile as tile
from concourse import mybir
from concourse.masks import make_identity
from concourse.bass_utils import run_bass_kernel_spmd

B, T, D_MEM, D_Q, D_ATT, C_LOC, KW = 64, 2000, 512, 1024, 128, 32, 31
PAD = KW // 2  # 15
TP = 2048  # padded T
N_CORES = 8
BPC = B // N_CORES  # 8 batches per core
NCHUNK = 4
TCH = TP // NCHUNK  # 512
NSUB = 16
PP = 128  # partitions; t = 128*g + p
KO = D_Q // 128

f32 = mybir.dt.float32
f32r = mybir.dt.float32r

CTX_F32R = os.environ.get("KERNEL_CTX_F32R", "1") == "1"
PL_F32R = os.environ.get("KERNEL_PL_F32R", "1") == "1"


def _build_nc():
    nc = bacc.Bacc("TRN2", target_bir_lowering=False)

    hid_h = nc.dram_tensor("hid", [BPC, D_Q], f32, kind="ExternalInput")
    mem_h = nc.dram_tensor("mem", [BPC, TP, D_MEM], f32, kind="ExternalInput")
    pm_h = nc.dram_tensor("pm", [BPC, TP, D_ATT], f32, kind="ExternalInput")
    aw_h = nc.dram_tensor("aw", [BPC * 2, T], f32, kind="ExternalInput")
    wq_h = nc.dram_tensor("wq", [D_Q, D_ATT], f32, kind="ExternalInput")
    wconv_h = nc.dram_tensor("wconv", [C_LOC, 2 * KW], f32, kind="ExternalInput")
    bconv_h = nc.dram_tensor("bconv", [C_LOC], f32, kind="ExternalInput")
    wloc_h = nc.dram_tensor("wloc", [C_LOC, D_ATT], f32, kind="ExternalInput")
    v_h = nc.dram_tensor("v", [D_ATT], f32, kind="ExternalInput")

    ctx_h = nc.dram_tensor("ctx_out", [BPC, D_MEM], f32, kind="ExternalOutput")
    w_h = nc.dram_tensor("w_out", [BPC, T], f32, kind="ExternalOutput")

    with tile.TileContext(nc) as tc, ExitStack() as ctx:
        _body(tc, ctx, hid_h, mem_h, pm_h, aw_h, wq_h, wconv_h, bconv_h,
              wloc_h, v_h, ctx_h, w_h)
    nc.compile()
    return nc


def _body(tc, ctx, hid_h, mem_h, pm_h, aw_h, wq_h, wconv_h, bconv_h, wloc_h,
          v_h, ctx_h, w_h):
    nc = tc.nc
    AF = mybir.ActivationFunctionType
    sh_dt = f32r if PL_F32R else f32
    mem_dt = f32r if CTX_F32R else f32

    singles = ctx.enter_context(tc.tile_pool(name="singles", bufs=1))

    ident = singles.tile([128, 128], f32)
    make_identity(nc, ident)

    # --- one-time weight staging -----------------------------------------
    wcat = singles.tile([C_LOC, 62], f32)
    nc.gpsimd.dma_start(out=wcat, in_=wconv_h[:, :])
    bconv_sb = singles.tile([C_LOC, 1], f32)
    nc.gpsimd.dma_start(out=bconv_sb,
                        in_=bconv_h[:].rearrange("(c o) -> c o", o=1))
    wloc_sb = singles.tile([C_LOC, D_ATT], f32)
    nc.gpsimd.dma_start(out=wloc_sb, in_=wloc_h[:, :])
    v_sb = singles.tile([D_ATT, 1], f32)
    nc.gpsimd.dma_start(out=v_sb, in_=v_h[:].rearrange("(d o) -> d o", o=1))
    wq_sb = singles.tile([128, KO * D_ATT], f32)
    nc.gpsimd.dma_start(
        out=wq_sb.rearrange("p (ko n) -> p ko n", ko=KO),
        in_=wq_h[:, :].rearrange("(ko kp) n -> kp ko n", kp=128))
    hid_sb = singles.tile([BPC, D_Q], f32)
    nc.gpsimd.dma_start(out=hid_sb, in_=hid_h[:, :])

    ones128 = singles.tile([PP, 1], f32)
    nc.vector.memset(ones128, 1.0)
    # pad mask: 1.0 where t = 128*col + p < 2000, else 0.0
    padmask = singles.tile([PP, NSUB], f32)
    nc.gpsimd.memset(padmask, 1.0)
    nc.gpsimd.affine_select(
        out=padmask, in_=padmask, compare_op=mybir.AluOpType.is_ge,
        fill=0.0, base=T - 1, channel_multiplier=-1,
        pattern=[[-PP, NSUB]])
    ones16 = singles.tile([NSUB, NSUB], f32)
    nc.vector.memset(ones16, 1.0)

    # aw rows: batch pair (2b, 2b+1) on adjacent partitions, pairs spread
    # over distinct SBUF ports so the per-batch shift DMAs read in parallel.
    awall = singles.tile([128, TP + 2 * PAD], sh_dt)
    nc.vector.memset(awall[:, 0:PAD].bitcast(f32), 0.0)
    nc.vector.memset(awall[:, PAD + T:].bitcast(f32), 0.0)

    def _aw_part(i):
        b, ci = divmod(i, 2)
        return 8 * (b % 4) + 64 * (b // 4) + ci

    for i in range(2 * BPC):
        p = _aw_part(i)
        nc.gpsimd.dma_start(out=awall[p:p + 1, PAD:PAD + T].bitcast(f32),
                            in_=aw_h[i:i + 1, :])

    wcomb62 = singles.tile([62, D_ATT], sh_dt)
    qtb = singles.tile([D_ATT, BPC], f32)
    with tc.tile_pool(name="setup_ps", bufs=1, space="PSUM") as sps, \
         tc.tile_pool(name="setup_sb", bufs=2) as ssb:
        # Wcomb = wcat.T @ wloc -> [62, 128]; bconv folds into the tanh
        # bias as Wloc.T @ bconv.
        wcomb_ps = sps.tile([62, D_ATT], f32, tag="wc")
        nc.tensor.matmul(wcomb_ps, lhsT=wcat, rhs=wloc_sb, start=True,
                         stop=True)
        nc.vector.tensor_copy(wcomb62, wcomb_ps)
        bias_ps = sps.tile([D_ATT, 1], f32, tag="bias")
        nc.tensor.matmul(bias_ps, lhsT=wloc_sb, rhs=bconv_sb, start=True,
                         stop=True)
        biaspl = ssb.tile([D_ATT, 1], f32, tag="biaspl")
        nc.vector.tensor_copy(biaspl, bias_ps)

        # hidT [1024, 8] via PE transposes, then qT = Wq.T @ hid.T  [128, 8]
        hidT = ssb.tile([128, KO * BPC], f32, tag="hidT")
        for ko in range(KO):
            ht_ps = sps.tile([128, BPC], f32, tag="ht")
            nc.tensor.transpose(ht_ps, hid_sb[:, 128 * ko:128 * (ko + 1)],
                                ident[0:BPC, 0:BPC])
            nc.vector.tensor_copy(hidT[:, BPC * ko:BPC * (ko + 1)], ht_ps)
        qt_ps = sps.tile([D_ATT, BPC], f32, tag="qt")
        for ko in range(KO):
            nc.tensor.matmul(qt_ps, lhsT=wq_sb[:, 128 * ko:128 * (ko + 1)],
                             rhs=hidT[:, BPC * ko:BPC * (ko + 1)],
                             start=(ko == 0), stop=(ko == KO - 1))
        nc.vector.tensor_scalar(out=qtb, in0=qt_ps, scalar1=biaspl,
                                scalar2=None, op0=mybir.AluOpType.add)

    # --- main pools -------------------------------------------------------
    shpool = ctx.enter_context(tc.tile_pool(name="sh", bufs=2))
    mempool = ctx.enter_context(tc.tile_pool(name="mem", bufs=2))
    pmpool = ctx.enter_context(tc.tile_pool(name="pm", bufs=1))
    thpool = ctx.enter_context(tc.tile_pool(name="th", bufs=3))
    ptpool = ctx.enter_context(tc.tile_pool(name="pt", bufs=2))
    wpool = ctx.enter_context(tc.tile_pool(name="wout", bufs=2))
    ctxsb = ctx.enter_context(tc.tile_pool(name="ctxsb", bufs=2))
    smallsb = ctx.enter_context(tc.tile_pool(name="smallsb", bufs=2))

    p12pool = ctx.enter_context(tc.tile_pool(name="p12", bufs=2, space="PSUM"))
    etpool = ctx.enter_context(tc.tile_pool(name="et", bufs=2, space="PSUM"))
    ctxpool = ctx.enter_context(tc.tile_pool(name="ctxp", bufs=2, space="PSUM"))
    tinyps = ctx.enter_context(tc.tile_pool(name="tinyps", bufs=1, space="PSUM"))

    # ALL of processed_memory is loaded upfront on the scalar-engine HWDGE
    # queue (it fits in SBUF at 64KB/partition), so the per-batch critical
    # path only waits on its own mem tile from the sync queue.
    pm_all = pmpool.tile([PP, BPC * NSUB * D_ATT], f32)
    for h in range(BPC):
        nc.scalar.dma_start(
            out=pm_all.rearrange("p (b g n) -> p b g n", b=BPC, g=NSUB)[
                :, h:h + 1],
            in_=pm_h[h:h + 1].rearrange("b (g p) n -> p b g n", p=PP))

    for b in range(BPC):
        mem1 = mempool.tile([PP, NSUB * D_MEM], mem_dt)
        src = mem_h[b].rearrange("(g p) n -> p g n", p=PP)
        if CTX_F32R:
            src = src.bitcast(f32r)
        nc.sync.dma_start(
            out=mem1.rearrange("p (g n) -> p g n", g=NSUB), in_=src)

        def mem_g(g):
            return mem1[:, g * D_MEM:(g + 1) * D_MEM]

        def pm_g(g):
            return pm_all[:, (b * NSUB + g) * D_ATT:(b * NSUB + g + 1) * D_ATT]

        # shifted windows: sh[31*ci + k, t] = aw[b, ci, t + k - 15]
        sh = shpool.tile([62, TP], sh_dt)
        p0 = _aw_part(2 * b)
        row = awall[p0:p0 + 2, 0:1]
        src = bass.AP(tensor=row.tensor, offset=row.offset,
                      ap=[list(row.ap[0]), [1, KW], [1, TP]])
        nc.gpsimd.dma_start(out=sh, in_=src)

        eT_ps = etpool.tile([PP, NSUB], f32)
        for c in range(NCHUNK):
            # pl matmul first (start=True zeroes the whole PSUM bank),
            # then the pm^T PE transposes accumulate on top (start=False).
            p12 = p12pool.tile([D_ATT, TCH], f32)
            nc.tensor.matmul(p12, lhsT=wcomb62,
                             rhs=sh[:, TCH * c:TCH * (c + 1)],
                             start=True, stop=False, skip_group_check=True)
            for j in range(4):
                g = 4 * c + j
                nc.tensor.matmul(p12[:, PP * j:PP * (j + 1)],
                                 lhsT=pm_g(g), rhs=ident,
                                 start=False, stop=(j == 3),
                                 is_transpose=True, skip_group_check=True)
            th = thpool.tile([D_ATT, TCH], f32)
            nc.scalar.activation(out=th, in_=p12, func=AF.Tanh,
                                 bias=qtb[:, b:b + 1], scale=1.0)
            for j in range(4):
                g = 4 * c + j
                nc.tensor.matmul(eT_ps[:, g:g + 1],
                                 lhsT=th[:, PP * j:PP * (j + 1)],
                                 rhs=v_sb, start=(g == 0), stop=(g == 15),
                                 skip_group_check=True)

        # exp, with the 48 padded t slots (t >= 2000) masked to zero
        pt_raw = ptpool.tile([PP, NSUB], f32, tag="ptraw")
        nc.scalar.activation(out=pt_raw, in_=eT_ps, func=AF.Exp, scale=1.0)
        pt_sb = ptpool.tile([PP, NSUB], f32, tag="pt")
        nc.vector.tensor_mul(pt_sb, pt_raw, padmask)
        if CTX_F32R:
            pt_mm = ptpool.tile([PP, NSUB], f32r, tag="ptr")
            nc.vector.tensor_copy(pt_mm, pt_sb)
        else:
            pt_mm = pt_sb

        ctx_ps = ctxpool.tile([1, D_MEM], f32)
        for g in range(NSUB):
            nc.tensor.matmul(ctx_ps, lhsT=pt_mm[:, g:g + 1], rhs=mem_g(g),
                             start=(g == 0), stop=(g == NSUB - 1),
                             skip_group_check=True)

        # softmax normalization
        p16_ps = tinyps.tile([NSUB, PP], f32, tag="p16")
        nc.tensor.matmul(p16_ps, lhsT=pt_sb, rhs=ident, start=True, stop=True,
                         is_transpose=True)
        cs_ps = tinyps.tile([NSUB, 1], f32, tag="tiny")
        nc.tensor.matmul(cs_ps, lhsT=pt_sb, rhs=ones128, start=True, stop=True)
        cs_sb = smallsb.tile([NSUB, 1], f32, tag="cs")
        nc.vector.tensor_copy(cs_sb, cs_ps)
        sr_ps = tinyps.tile([NSUB, 1], f32, tag="tiny")
        nc.tensor.matmul(sr_ps, lhsT=ones16, rhs=cs_sb, start=True, stop=True)
        rs_sb = smallsb.tile([NSUB, 1], f32, tag="rs")
        nc.vector.reciprocal(rs_sb, sr_ps)

        # outputs go out on the scalar-engine HWDGE queue so they never
        # block the sync-engine load queue.
        w_sb = wpool.tile([NSUB, PP], f32)
        nc.vector.tensor_scalar_mul(out=w_sb, in0=p16_ps, scalar1=rs_sb)
        nc.scalar.dma_start(
            out=w_h[b, 0:15 * PP].rearrange("(g p) -> g p", g=15),
            in_=w_sb[0:15, :])
        nc.scalar.dma_start(
            out=w_h[b, 15 * PP:T].rearrange("(o p) -> o p", o=1),
            in_=w_sb[15:16, 0:T - 15 * PP])

        ctx_sb = ctxsb.tile([1, D_MEM], f32)
        nc.vector.tensor_scalar_mul(out=ctx_sb, in0=ctx_ps,
                                    scalar1=rs_sb[0:1, :])
        nc.scalar.dma_start(out=ctx_h[b].rearrange("(o n) -> o n", o=1),
                            in_=ctx_sb)


_NC_CACHE = {}


def _get_nc():
    key = (CTX_F32R, PL_F32R)
    if key not in _NC_CACHE:
        _NC_CACHE[key] = _build_nc()
    return _NC_CACHE[key]


def _pad_t(x, tp):
    out = np.zeros(x.shape[:-2] + (tp, x.shape[-1]), np.float32)
    out[..., :x.shape[-2], :] = x
    return out


def _make_in_maps(inputs):
    hid = np.ascontiguousarray(
        np.asarray(inputs["attention_hidden_state"], np.float32).reshape(B, D_Q))
    mem = _pad_t(np.asarray(inputs["memory"], np.float32), TP)
    pm = _pad_t(np.asarray(inputs["processed_memory"], np.float32), TP)
    aw = np.ascontiguousarray(np.asarray(inputs["attention_weights"], np.float32))
    wq = np.ascontiguousarray(np.asarray(inputs["Wq"], np.float32))
    wconv = np.ascontiguousarray(
        np.asarray(inputs["Wconv"], np.float32).reshape(C_LOC, 2 * KW))
    bconv = np.ascontiguousarray(np.asarray(inputs["bconv"], np.float32))
    wloc = np.ascontiguousarray(np.asarray(inputs["Wloc"], np.float32))
    v = np.ascontiguousarray(np.asarray(inputs["v"], np.float32))

    in_maps = []
    for c in range(N_CORES):
        s = slice(BPC * c, BPC * (c + 1))
        in_maps.append({
            "hid": hid[s],
            "mem": mem[s],
            "pm": pm[s],
            "aw": np.ascontiguousarray(aw[s].reshape(BPC * 2, T)),
            "wq": wq,
            "wconv": wconv,
            "bconv": bconv,
            "wloc": wloc,
            "v": v,
        })
    return in_maps


def run(inputs, trace=False):
    nc = _get_nc()
    in_maps = _make_in_maps(inputs)
    res = run_bass_kernel_spmd(nc, in_maps, core_ids=list(range(N_CORES)),
                               trace=trace)
    ctx = np.concatenate([res.results[c]["ctx_out"] for c in range(N_CORES)], 0)
    w = np.concatenate([res.results[c]["w_out"] for c in range(N_CORES)], 0)
    return (ctx, w), res


def kernel(**inputs):
    (ctx, w), _ = run(inputs, trace=False)
    return ctx, w


if __name__ == "__main__":
    nc = _get_nc()
    print("built ok")


# revision 20
# speedup vs baseline: 1.0290x; 1.0290x over previous
"""Trainium2 Bass kernel: Tacotron-style location-sensitive attention.

reference math (per batch b):
  q      = hidden[b] @ Wq                              [1, 128]
  loc    = conv1d(aw[b], Wconv, pad=15) + bconv        [32, T]
  pl     = loc.T @ Wloc                                [T, 128]
  e[t]   = v . tanh(q + pl[t] + pm[b,t])               [T]
  p      = softmax(e)                                  [T]
  ctx    = p @ mem[b]                                  [512]

Sharding: pure data parallel, 8 batches per core on 8 cores.

Layout (per core, 8 batches). T is host-padded 2000->2048 so every on-chip
tile uses the full 128 partitions (125-partition DMAs measure 2.3x slower).
t factors as t = 128*g + p (group g in [0,16), partition p in [0,128)):
  - conv + location projection collapse into ONE matmul per 512-col chunk:
      pl^T[d, t] = Wcomb[0:62].T @ shifted[62, t]
    with shifted[31*ci+k, t] = aw[ci, t+k-15] built by an overlapped-window
    SBUF->SBUF DMA, and Wcomb = Wconv_flat.T @ Wloc computed on device once.
    The bconv term (bconv @ Wloc) and the query q fold into the tanh bias
    (d on partitions).
  - pm tiles are PE-transposed and accumulated into the same PSUM tile as
    the pl matmul (pl first with start=True, transposes start=False).
  - e^T columns [128,1] via PE (lhsT = tanh tile, rhs = v), exp on ACT,
    the 48 padded t slots are zeroed in p, context = 16 accumulating
    matmuls streaming mem tiles [128, 512].
  - softmax norm: column sums via PE + ones, replicated total via a
    ones[16,16] matmul, reciprocal on DVE, scale + DMA out on the scalar
    (ACT) HWDGE queue so stores never block the sync-load queue.
"""

import os
import sys

sys.path.insert(0, "/opt/trn_rl_repo")

import numpy as np
from contextlib import ExitStack

import concourse.bass as bass
import concourse.bacc as bacc
import concourse.tile as tile
from concourse import mybir
from concourse.masks import make_identity
from concourse.bass_utils import run_bass_kernel_spmd

B, T, D_MEM, D_Q, D_ATT, C_LOC, KW = 64, 2000, 512, 1024, 128, 32, 31
PAD = KW // 2  # 15
TP = 2048  # padded T
N_CORES = 8
BPC = B // N_CORES  # 8 batches per core
NCHUNK = 4
TCH = TP // NCHUNK  # 512
NSUB = 16
PP = 128  # partitions; t = 128*g + p
KO = D_Q // 128

f32 = mybir.dt.float32
f32r = mybir.dt.float32r

CTX_F32R = os.environ.get("KERNEL_CTX_F32R", "1") == "1"
PL_F32R = os.environ.get("KERNEL_PL_F32R", "1") == "1"


def _build_nc():
    nc = bacc.Bacc("TRN2", target_bir_lowering=False)

    hid_h = nc.dram_tensor("hid", [BPC, D_Q], f32, kind="ExternalInput")
    mem_h = nc.dram_tensor("mem", [BPC, PP, NSUB * D_MEM], f32,
                           kind="ExternalInput")
    pm_h = nc.dram_tensor("pm", [BPC, D_ATT, TP], f32, kind="ExternalInput")
    aw_h = nc.dram_tensor("aw", [BPC * 2, T], f32, kind="ExternalInput")
    wq_h = nc.dram_tensor("wq", [D_Q, D_ATT], f32, kind="ExternalInput")
    wconv_h = nc.dram_tensor("wconv", [C_LOC, 2 * KW], f32, kind="ExternalInput")
    bconv_h = nc.dram_tensor("bconv", [C_LOC], f32, kind="ExternalInput")
    wloc_h = nc.dram_tensor("wloc", [C_LOC, D_ATT], f32, kind="ExternalInput")
    v_h = nc.dram_tensor("v", [D_ATT], f32, kind="ExternalInput")

    ctx_h = nc.dram_tensor("ctx_out", [BPC, D_MEM], f32, kind="ExternalOutput")
    w_h = nc.dram_tensor("w_out", [BPC, T], f32, kind="ExternalOutput")

    with tile.TileContext(nc) as tc, ExitStack() as ctx:
        _body(tc, ctx, hid_h, mem_h, pm_h, aw_h, wq_h, wconv_h, bconv_h,
              wloc_h, v_h, ctx_h, w_h)
    nc.compile()
    return nc


def _body(tc, ctx, hid_h, mem_h, pm_h, aw_h, wq_h, wconv_h, bconv_h, wloc_h,
          v_h, ctx_h, w_h):
    nc = tc.nc
    AF = mybir.ActivationFunctionType
    sh_dt = f32r if PL_F32R else f32
    mem_dt = f32r if CTX_F32R else f32

    singles = ctx.enter_context(tc.tile_pool(name="singles", bufs=1))

    ident = singles.tile([128, 128], f32)
    make_identity(nc, ident)

    # --- one-time weight staging -----------------------------------------
    wcat = singles.tile([C_LOC, 62], f32)
    nc.gpsimd.dma_start(out=wcat, in_=wconv_h[:, :])
    bconv_sb = singles.tile([C_LOC, 1], f32)
    nc.gpsimd.dma_start(out=bconv_sb,
                        in_=bconv_h[:].rearrange("(c o) -> c o", o=1))
    wloc_sb = singles.tile([C_LOC, D_ATT], f32)
    nc.gpsimd.dma_start(out=wloc_sb, in_=wloc_h[:, :])
    v_sb = singles.tile([D_ATT, 1], f32)
    nc.gpsimd.dma_start(out=v_sb, in_=v_h[:].rearrange("(d o) -> d o", o=1))
    wq_sb = singles.tile([128, KO * D_ATT], f32)
    nc.gpsimd.dma_start(
        out=wq_sb.rearrange("p (ko n) -> p ko n", ko=KO),
        in_=wq_h[:, :].rearrange("(ko kp) n -> kp ko n", kp=128))
    hid_sb = singles.tile([BPC, D_Q], f32)
    nc.gpsimd.dma_start(out=hid_sb, in_=hid_h[:, :])

    ones128 = singles.tile([PP, 1], f32)
    nc.vector.memset(ones128, 1.0)
    # pad mask: 1.0 where t = 128*col + p < 2000, else 0.0
    padmask = singles.tile([PP, NSUB], f32)
    nc.gpsimd.memset(padmask, 1.0)
    nc.gpsimd.affine_select(
        out=padmask, in_=padmask, compare_op=mybir.AluOpType.is_ge,
        fill=0.0, base=T - 1, channel_multiplier=-1,
        pattern=[[-PP, NSUB]])
    ones16 = singles.tile([NSUB, NSUB], f32)
    nc.vector.memset(ones16, 1.0)

    # aw rows: batch pair (2b, 2b+1) on adjacent partitions, pairs spread
    # over distinct SBUF ports so the per-batch shift DMAs read in parallel.
    awall = singles.tile([128, TP + 2 * PAD], sh_dt)
    nc.vector.memset(awall[:, 0:PAD].bitcast(f32), 0.0)
    nc.vector.memset(awall[:, PAD + T:].bitcast(f32), 0.0)

    def _aw_part(i):
        b, ci = divmod(i, 2)
        return 8 * (b % 4) + 64 * (b // 4) + ci

    for i in range(2 * BPC):
        p = _aw_part(i)
        nc.gpsimd.dma_start(out=awall[p:p + 1, PAD:PAD + T].bitcast(f32),
                            in_=aw_h[i:i + 1, :])

    wcomb62 = singles.tile([62, D_ATT], sh_dt)
    qtb = singles.tile([D_ATT, BPC], f32)
    with tc.tile_pool(name="setup_ps", bufs=1, space="PSUM") as sps, \
         tc.tile_pool(name="setup_sb", bufs=2) as ssb:
        # Wcomb = wcat.T @ wloc -> [62, 128]; bconv folds into the tanh
        # bias as Wloc.T @ bconv.
        wcomb_ps = sps.tile([62, D_ATT], f32, tag="wc")
        nc.tensor.matmul(wcomb_ps, lhsT=wcat, rhs=wloc_sb, start=True,
                         stop=True)
        nc.vector.tensor_copy(wcomb62, wcomb_ps)
        bias_ps = sps.tile([D_ATT, 1], f32, tag="bias")
        nc.tensor.matmul(bias_ps, lhsT=wloc_sb, rhs=bconv_sb, start=True,
                         stop=True)
        biaspl = ssb.tile([D_ATT, 1], f32, tag="biaspl")
        nc.vector.tensor_copy(biaspl, bias_ps)

        # hidT [1024, 8] via PE transposes, then qT = Wq.T @ hid.T  [128, 8]
        hidT = ssb.tile([128, KO * BPC], f32, tag="hidT")
        for ko in range(KO):
            ht_ps = sps.tile([128, BPC], f32, tag="ht")
            nc.tensor.transpose(ht_ps, hid_sb[:, 128 * ko:128 * (ko + 1)],
                                ident[0:BPC, 0:BPC])
            nc.vector.tensor_copy(hidT[:, BPC * ko:BPC * (ko + 1)], ht_ps)
        qt_ps = sps.tile([D_ATT, BPC], f32, tag="qt")
        for ko in range(KO):
            nc.tensor.matmul(qt_ps, lhsT=wq_sb[:, 128 * ko:128 * (ko + 1)],
                             rhs=hidT[:, BPC * ko:BPC * (ko + 1)],
                             start=(ko == 0), stop=(ko == KO - 1))
        nc.vector.tensor_scalar(out=qtb, in0=qt_ps, scalar1=biaspl,
                                scalar2=None, op0=mybir.AluOpType.add)

    # --- main pools -------------------------------------------------------
    shpool = ctx.enter_context(tc.tile_pool(name="sh", bufs=2))
    mempool = ctx.enter_context(tc.tile_pool(name="mem", bufs=2))
    pmpool = ctx.enter_context(tc.tile_pool(name="pm", bufs=1))
    thpool = ctx.enter_context(tc.tile_pool(name="th", bufs=3))
    ptpool = ctx.enter_context(tc.tile_pool(name="pt", bufs=2))
    wpool = ctx.enter_context(tc.tile_pool(name="wout", bufs=2))
    ctxsb = ctx.enter_context(tc.tile_pool(name="ctxsb", bufs=2))
    smallsb = ctx.enter_context(tc.tile_pool(name="smallsb", bufs=2))

    p12pool = ctx.enter_context(tc.tile_pool(name="p12", bufs=2, space="PSUM"))
    etpool = ctx.enter_context(tc.tile_pool(name="et", bufs=2, space="PSUM"))
    ctxpool = ctx.enter_context(tc.tile_pool(name="ctxp", bufs=2, space="PSUM"))
    tinyps = ctx.enter_context(tc.tile_pool(name="tinyps", bufs=1, space="PSUM"))

    # ALL of processed_memory is loaded upfront on the scalar-engine HWDGE
    # queue (it fits in SBUF at 64KB/partition), so the per-batch critical
    # path only waits on its own mem tile from the sync queue.
    # processed_memory arrives host-transposed [d, t]; whole-core copy fits
    # in SBUF (64KB/partition) and loads as contiguous 8KB runs per
    # partition on the scalar HWDGE queue.
    pm_all = pmpool.tile([D_ATT, BPC * TP], f32)
    for h in range(BPC):
        nc.scalar.dma_start(out=pm_all[:, h * TP:(h + 1) * TP],
                            in_=pm_h[h])

    for b in range(BPC):
        mem1 = mempool.tile([PP, NSUB * D_MEM], mem_dt)
        src = mem_h[b]
        if CTX_F32R:
            src = src.bitcast(f32r)
        nc.sync.dma_start(out=mem1, in_=src)

        def mem_g(g):
            return mem1[:, g * D_MEM:(g + 1) * D_MEM]

        # shifted windows: sh[31*ci + k, t] = aw[b, ci, t + k - 15]
        sh = shpool.tile([62, TP], sh_dt)
        p0 = _aw_part(2 * b)
        row = awall[p0:p0 + 2, 0:1]
        src = bass.AP(tensor=row.tensor, offset=row.offset,
                      ap=[list(row.ap[0]), [1, KW], [1, TP]])
        nc.gpsimd.dma_start(out=sh, in_=src)

        eT_ps = etpool.tile([PP, NSUB], f32)
        for c in range(NCHUNK):
            p12 = p12pool.tile([D_ATT, TCH], f32)
            nc.tensor.matmul(p12, lhsT=wcomb62,
                             rhs=sh[:, TCH * c:TCH * (c + 1)],
                             start=True, stop=True)
            t1 = thpool.tile([D_ATT, TCH], f32, tag="t1")
            nc.vector.tensor_add(
                t1, p12, pm_all[:, b * TP + TCH * c:b * TP + TCH * (c + 1)])
            th = thpool.tile([D_ATT, TCH], f32, tag="th")
            nc.scalar.activation(out=th, in_=t1, func=AF.Tanh,
                                 bias=qtb[:, b:b + 1], scale=1.0)
            for j in range(4):
                g = 4 * c + j
                nc.tensor.matmul(eT_ps[:, g:g + 1],
                                 lhsT=th[:, PP * j:PP * (j + 1)],
                                 rhs=v_sb, start=(g == 0), stop=(g == 15),
                                 skip_group_check=True)

        # exp, with the 48 padded t slots (t >= 2000) masked to zero
        pt_raw = ptpool.tile([PP, NSUB], f32, tag="ptraw")
        nc.scalar.activation(out=pt_raw, in_=eT_ps, func=AF.Exp, scale=1.0)
        pt_sb = ptpool.tile([PP, NSUB], f32, tag="pt")
        nc.vector.tensor_mul(pt_sb, pt_raw, padmask)
        if CTX_F32R:
            pt_mm = ptpool.tile([PP, NSUB], f32r, tag="ptr")
            nc.vector.tensor_copy(pt_mm, pt_sb)
        else:
            pt_mm = pt_sb

        ctx_ps = ctxpool.tile([1, D_MEM], f32)
        for g in range(NSUB):
            nc.tensor.matmul(ctx_ps, lhsT=pt_mm[:, g:g + 1], rhs=mem_g(g),
                             start=(g == 0), stop=(g == NSUB - 1),
                             skip_group_check=True)

        # softmax normalization
        p16_ps = tinyps.tile([NSUB, PP], f32, tag="p16")
        nc.tensor.matmul(p16_ps, lhsT=pt_sb, rhs=ident, start=True, stop=True,
                         is_transpose=True)
        cs_ps = tinyps.tile([NSUB, 1], f32, tag="tiny")
        nc.tensor.matmul(cs_ps, lhsT=pt_sb, rhs=ones128, start=True, stop=True)
        cs_sb = smallsb.tile([NSUB, 1], f32, tag="cs")
        nc.vector.tensor_copy(cs_sb, cs_ps)
        sr_ps = tinyps.tile([NSUB, 1], f32, tag="tiny")
        nc.tensor.matmul(sr_ps, lhsT=ones16, rhs=cs_sb, start=True, stop=True)
        rs_sb = smallsb.tile([NSUB, 1], f32, tag="rs")
        nc.vector.reciprocal(rs_sb, sr_ps)

        # outputs go out on the scalar-engine HWDGE queue so they never
        # block the sync-engine load queue.
        w_sb = wpool.tile([NSUB, PP], f32)
        nc.vector.tensor_scalar_mul(out=w_sb, in0=p16_ps, scalar1=rs_sb)
        nc.scalar.dma_start(
            out=w_h[b, 0:15 * PP].rearrange("(g p) -> g p", g=15),
            in_=w_sb[0:15, :])
        nc.scalar.dma_start(
            out=w_h[b, 15 * PP:T].rearrange("(o p) -> o p", o=1),
            in_=w_sb[15:16, 0:T - 15 * PP])

        ctx_sb = ctxsb.tile([1, D_MEM], f32)
        nc.vector.tensor_scalar_mul(out=ctx_sb, in0=ctx_ps,
                                    scalar1=rs_sb[0:1, :])
        nc.scalar.dma_start(out=ctx_h[b].rearrange("(o n) -> o n", o=1),
                            in_=ctx_sb)


_NC_CACHE = {}


def _get_nc():
    key = (CTX_F32R, PL_F32R)
    if key not in _NC_CACHE:
        _NC_CACHE[key] = _build_nc()
    return _NC_CACHE[key]


def _pad_t(x, tp):
    out = np.zeros(x.shape[:-2] + (tp, x.shape[-1]), np.float32)
    out[..., :x.shape[-2], :] = x
    return out


def _marshal_mem(mem):
    # [B, TP, 512] -> [B, 128, 16*512] with t = 128*g + p
    m = _pad_t(mem, TP).reshape(B, NSUB, PP, D_MEM)
    return np.ascontiguousarray(m.transpose(0, 2, 1, 3).reshape(B, PP, NSUB * D_MEM))


def _marshal_pm(pm):
    # [B, TP, 128] -> [B, 128, TP] (d on partitions)
    p = _pad_t(pm, TP)
    return np.ascontiguousarray(p.transpose(0, 2, 1))


def _make_in_maps(inputs):
    hid = np.ascontiguousarray(
        np.asarray(inputs["attention_hidden_state"], np.float32).reshape(B, D_Q))
    mem = _marshal_mem(np.asarray(inputs["memory"], np.float32))
    pm = _marshal_pm(np.asarray(inputs["processed_memory"], np.float32))
    aw = np.ascontiguousarray(np.asarray(inputs["attention_weights"], np.float32))
    wq = np.ascontiguousarray(np.asarray(inputs["Wq"], np.float32))
    wconv = np.ascontiguousarray(
        np.asarray(inputs["Wconv"], np.float32).reshape(C_LOC, 2 * KW))
    bconv = np.ascontiguousarray(np.asarray(inputs["bconv"], np.float32))
    wloc = np.ascontiguousarray(np.asarray(inputs["Wloc"], np.float32))
    v = np.ascontiguousarray(np.asarray(inputs["v"], np.float32))

    in_maps = []
    for c in range(N_CORES):
        s = slice(BPC * c, BPC * (c + 1))
        in_maps.append({
            "hid": hid[s],
            "mem": mem[s],
            "pm": pm[s],
            "aw": np.ascontiguousarray(aw[s].reshape(BPC * 2, T)),
            "wq": wq,
            "wconv": wconv,
            "bconv": bconv,
            "wloc": wloc,
            "v": v,
        })
    return in_maps


def run(inputs, trace=False):
    nc = _get_nc()
    in_maps = _make_in_maps(inputs)
    res = run_bass_kernel_spmd(nc, in_maps, core_ids=list(range(N_CORES)),
                               trace=trace)
    ctx = np.concatenate([res.results[c]["ctx_out"] for c in range(N_CORES)], 0)
    w = np.concatenate([res.results[c]["w_out"] for c in range(N_CORES)], 0)
    return (ctx, w), res


def kernel(**inputs):
    (ctx, w), _ = run(inputs, trace=False)
    return ctx, w


if __name__ == "__main__":
    nc = _get_nc()
    print("built ok")


# revision 22
# speedup vs baseline: 1.0331x; 1.0040x over previous
"""Trainium2 Bass kernel: Tacotron-style location-sensitive attention.

reference math (per batch b):
  q      = hidden[b] @ Wq                              [1, 128]
  loc    = conv1d(aw[b], Wconv, pad=15) + bconv        [32, T]
  pl     = loc.T @ Wloc                                [T, 128]
  e[t]   = v . tanh(q + pl[t] + pm[b,t])               [T]
  p      = softmax(e)                                  [T]
  ctx    = p @ mem[b]                                  [512]

Sharding: pure data parallel, 8 batches per core on 8 cores.

Layout (per core, 8 batches). T is host-padded 2000->2048 so every on-chip
tile uses the full 128 partitions (125-partition DMAs measure 2.3x slower).
t factors as t = 128*g + p (group g in [0,16), partition p in [0,128)):
  - conv + location projection collapse into ONE matmul per 512-col chunk:
      pl^T[d, t] = Wcomb[0:62].T @ shifted[62, t]
    with shifted[31*ci+k, t] = aw[ci, t+k-15] built by an overlapped-window
    SBUF->SBUF DMA, and Wcomb = Wconv_flat.T @ Wloc computed on device once.
    The bconv term (bconv @ Wloc) and the query q fold into the tanh bias
    (d on partitions).
  - pm tiles are PE-transposed and accumulated into the same PSUM tile as
    the pl matmul (pl first with start=True, transposes start=False).
  - e^T columns [128,1] via PE (lhsT = tanh tile, rhs = v), exp on ACT,
    the 48 padded t slots are zeroed in p, context = 16 accumulating
    matmuls streaming mem tiles [128, 512].
  - softmax norm: column sums via PE + ones, replicated total via a
    ones[16,16] matmul, reciprocal on DVE, scale + DMA out on the scalar
    (ACT) HWDGE queue so stores never block the sync-load queue.
"""

import os
import sys

sys.path.insert(0, "/opt/trn_rl_repo")

import numpy as np
from contextlib import ExitStack

import concourse.bass as bass
import concourse.bacc as bacc
import concourse.tile as tile
from concourse import mybir
from concourse.masks import make_identity
from concourse.bass_utils import run_bass_kernel_spmd

B, T, D_MEM, D_Q, D_ATT, C_LOC, KW = 64, 2000, 512, 1024, 128, 32, 31
PAD = KW // 2  # 15
TP = 2048  # padded T
N_CORES = 8
BPC = B // N_CORES  # 8 batches per core
NCHUNK = 4
TCH = TP // NCHUNK  # 512
NSUB = 16
PP = 128  # partitions; t = 128*g + p
KO = D_Q // 128

f32 = mybir.dt.float32
f32r = mybir.dt.float32r

CTX_F32R = os.environ.get("KERNEL_CTX_F32R", "1") == "1"
PL_F32R = os.environ.get("KERNEL_PL_F32R", "1") == "1"


def _build_nc():
    nc = bacc.Bacc("TRN2", target_bir_lowering=False)

    hid_h = nc.dram_tensor("hid", [BPC, D_Q], f32, kind="ExternalInput")
    mem_h = nc.dram_tensor("mem", [BPC, PP, NSUB * D_MEM], f32,
                           kind="ExternalInput")
    pm_h = nc.dram_tensor("pm", [BPC, D_ATT, TP], f32, kind="ExternalInput")
    aw_h = nc.dram_tensor("aw", [BPC * 2, PAD + TP + KW], f32,
                          kind="ExternalInput")
    wq_h = nc.dram_tensor("wq", [D_Q, D_ATT], f32, kind="ExternalInput")
    wconv_h = nc.dram_tensor("wconv", [C_LOC, 2 * KW], f32, kind="ExternalInput")
    bconv_h = nc.dram_tensor("bconv", [C_LOC], f32, kind="ExternalInput")
    wloc_h = nc.dram_tensor("wloc", [C_LOC, D_ATT], f32, kind="ExternalInput")
    v_h = nc.dram_tensor("v", [D_ATT], f32, kind="ExternalInput")

    ctx_h = nc.dram_tensor("ctx_out", [BPC, D_MEM], f32, kind="ExternalOutput")
    w_h = nc.dram_tensor("w_out", [BPC, T], f32, kind="ExternalOutput")

    with tile.TileContext(nc) as tc, ExitStack() as ctx:
        _body(tc, ctx, hid_h, mem_h, pm_h, aw_h, wq_h, wconv_h, bconv_h,
              wloc_h, v_h, ctx_h, w_h)
    nc.compile()
    return nc


def _body(tc, ctx, hid_h, mem_h, pm_h, aw_h, wq_h, wconv_h, bconv_h, wloc_h,
          v_h, ctx_h, w_h):
    nc = tc.nc
    AF = mybir.ActivationFunctionType
    sh_dt = f32r if PL_F32R else f32
    mem_dt = f32r if CTX_F32R else f32

    singles = ctx.enter_context(tc.tile_pool(name="singles", bufs=1))

    ident = singles.tile([128, 128], f32)
    make_identity(nc, ident)

    # --- one-time weight staging -----------------------------------------
    wcat = singles.tile([C_LOC, 62], f32)
    nc.gpsimd.dma_start(out=wcat, in_=wconv_h[:, :])
    bconv_sb = singles.tile([C_LOC, 1], f32)
    nc.gpsimd.dma_start(out=bconv_sb,
                        in_=bconv_h[:].rearrange("(c o) -> c o", o=1))
    wloc_sb = singles.tile([C_LOC, D_ATT], f32)
    nc.gpsimd.dma_start(out=wloc_sb, in_=wloc_h[:, :])
    v_sb = singles.tile([D_ATT, 1], f32)
    nc.gpsimd.dma_start(out=v_sb, in_=v_h[:].rearrange("(d o) -> d o", o=1))
    wq_sb = singles.tile([128, KO * D_ATT], f32)
    nc.gpsimd.dma_start(
        out=wq_sb.rearrange("p (ko n) -> p ko n", ko=KO),
        in_=wq_h[:, :].rearrange("(ko kp) n -> kp ko n", kp=128))
    hid_sb = singles.tile([BPC, D_Q], f32)
    nc.gpsimd.dma_start(out=hid_sb, in_=hid_h[:, :])

    ones128 = singles.tile([PP, 1], f32)
    nc.vector.memset(ones128, 1.0)
    # pad mask: 1.0 where t = 128*col + p < 2000, else 0.0
    padmask = singles.tile([PP, NSUB], f32)
    nc.gpsimd.memset(padmask, 1.0)
    nc.gpsimd.affine_select(
        out=padmask, in_=padmask, compare_op=mybir.AluOpType.is_ge,
        fill=0.0, base=T - 1, channel_multiplier=-1,
        pattern=[[-PP, NSUB]])
    ones16 = singles.tile([NSUB, NSUB], f32)
    nc.vector.memset(ones16, 1.0)

    wcomb62 = singles.tile([62, D_ATT], sh_dt)
    qtb = singles.tile([D_ATT, BPC], f32)
    with tc.tile_pool(name="setup_ps", bufs=1, space="PSUM") as sps, \
         tc.tile_pool(name="setup_sb", bufs=2) as ssb:
        # Wcomb = wcat.T @ wloc -> [62, 128]; bconv folds into the tanh
        # bias as Wloc.T @ bconv.
        wcomb_ps = sps.tile([62, D_ATT], f32, tag="wc")
        nc.tensor.matmul(wcomb_ps, lhsT=wcat, rhs=wloc_sb, start=True,
                         stop=True)
        nc.vector.tensor_copy(wcomb62, wcomb_ps)
        bias_ps = sps.tile([D_ATT, 1], f32, tag="bias")
        nc.tensor.matmul(bias_ps, lhsT=wloc_sb, rhs=bconv_sb, start=True,
                         stop=True)
        biaspl = ssb.tile([D_ATT, 1], f32, tag="biaspl")
        nc.vector.tensor_copy(biaspl, bias_ps)

        # hidT [1024, 8] via PE transposes, then qT = Wq.T @ hid.T  [128, 8]
        hidT = ssb.tile([128, KO * BPC], f32, tag="hidT")
        for ko in range(KO):
            ht_ps = sps.tile([128, BPC], f32, tag="ht")
            nc.tensor.transpose(ht_ps, hid_sb[:, 128 * ko:128 * (ko + 1)],
                                ident[0:BPC, 0:BPC])
            nc.vector.tensor_copy(hidT[:, BPC * ko:BPC * (ko + 1)], ht_ps)
        qt_ps = sps.tile([D_ATT, BPC], f32, tag="qt")
        for ko in range(KO):
            nc.tensor.matmul(qt_ps, lhsT=wq_sb[:, 128 * ko:128 * (ko + 1)],
                             rhs=hidT[:, BPC * ko:BPC * (ko + 1)],
                             start=(ko == 0), stop=(ko == KO - 1))
        nc.vector.tensor_scalar(out=qtb, in0=qt_ps, scalar1=biaspl,
                                scalar2=None, op0=mybir.AluOpType.add)

    # --- main pools -------------------------------------------------------
    shpool = ctx.enter_context(tc.tile_pool(name="sh", bufs=2))
    mempool = ctx.enter_context(tc.tile_pool(name="mem", bufs=2))
    pmpool = ctx.enter_context(tc.tile_pool(name="pm", bufs=1))
    thpool = ctx.enter_context(tc.tile_pool(name="th", bufs=3))
    ptpool = ctx.enter_context(tc.tile_pool(name="pt", bufs=2))
    wpool = ctx.enter_context(tc.tile_pool(name="wout", bufs=2))
    ctxsb = ctx.enter_context(tc.tile_pool(name="ctxsb", bufs=2))
    smallsb = ctx.enter_context(tc.tile_pool(name="smallsb", bufs=2))

    p12pool = ctx.enter_context(tc.tile_pool(name="p12", bufs=2, space="PSUM"))
    etpool = ctx.enter_context(tc.tile_pool(name="et", bufs=2, space="PSUM"))
    ctxpool = ctx.enter_context(tc.tile_pool(name="ctxp", bufs=2, space="PSUM"))
    tinyps = ctx.enter_context(tc.tile_pool(name="tinyps", bufs=1, space="PSUM"))

    # ALL of processed_memory is loaded upfront on the scalar-engine HWDGE
    # queue (it fits in SBUF at 64KB/partition), so the per-batch critical
    # path only waits on its own mem tile from the sync queue.
    # processed_memory arrives host-transposed [d, t]; whole-core copy fits
    # in SBUF (64KB/partition) and loads as contiguous 8KB runs per
    # partition on the scalar HWDGE queue.
    pm_all = pmpool.tile([D_ATT, BPC * TP], f32)
    for h in range(BPC):
        nc.scalar.dma_start(out=pm_all[:, h * TP:(h + 1) * TP],
                            in_=pm_h[h])

    for b in range(BPC):
        mem1 = mempool.tile([PP, NSUB * D_MEM], mem_dt)
        src = mem_h[b]
        if CTX_F32R:
            src = src.bitcast(f32r)
        nc.sync.dma_start(out=mem1, in_=src)

        def mem_g(g):
            return mem1[:, g * D_MEM:(g + 1) * D_MEM]

        # shifted windows straight from (host-padded) DRAM:
        # sh[31*ci + k, t] = aw[b, ci, t + k - 15], one overlapped-window DMA
        sh = shpool.tile([62, TP], sh_dt)
        rows = aw_h[2 * b:2 * b + 2, 0:1]
        if PL_F32R:
            rows = rows.bitcast(f32r)
        srcap = bass.AP(tensor=rows.tensor, offset=rows.offset,
                        ap=[list(rows.ap[0]), [1, KW], [1, TP]])
        nc.gpsimd.dma_start(out=sh, in_=srcap)

        eT_ps = etpool.tile([PP, NSUB], f32)
        for c in range(NCHUNK):
            p12 = p12pool.tile([D_ATT, TCH], f32)
            nc.tensor.matmul(p12, lhsT=wcomb62,
                             rhs=sh[:, TCH * c:TCH * (c + 1)],
                             start=True, stop=True)
            t1 = thpool.tile([D_ATT, TCH], f32, tag="t1")
            nc.vector.tensor_add(
                t1, p12, pm_all[:, b * TP + TCH * c:b * TP + TCH * (c + 1)])
            th = thpool.tile([D_ATT, TCH], f32, tag="th")
            nc.scalar.activation(out=th, in_=t1, func=AF.Tanh,
                                 bias=qtb[:, b:b + 1], scale=1.0)
            for j in range(4):
                g = 4 * c + j
                nc.tensor.matmul(eT_ps[:, g:g + 1],
                                 lhsT=th[:, PP * j:PP * (j + 1)],
                                 rhs=v_sb, start=(g == 0), stop=(g == 15),
                                 skip_group_check=True)

        # exp, with the 48 padded t slots (t >= 2000) masked to zero
        pt_raw = ptpool.tile([PP, NSUB], f32, tag="ptraw")
        nc.scalar.activation(out=pt_raw, in_=eT_ps, func=AF.Exp, scale=1.0)
        pt_sb = ptpool.tile([PP, NSUB], f32, tag="pt")
        nc.vector.tensor_mul(pt_sb, pt_raw, padmask)
        if CTX_F32R:
            pt_mm = ptpool.tile([PP, NSUB], f32r, tag="ptr")
            nc.vector.tensor_copy(pt_mm, pt_sb)
        else:
            pt_mm = pt_sb

        ctx_ps = ctxpool.tile([1, D_MEM], f32)
        for g in range(NSUB):
            nc.tensor.matmul(ctx_ps, lhsT=pt_mm[:, g:g + 1], rhs=mem_g(g),
                             start=(g == 0), stop=(g == NSUB - 1),
                             skip_group_check=True)

        # softmax normalization
        p16_ps = tinyps.tile([NSUB, PP], f32, tag="p16")
        nc.tensor.matmul(p16_ps, lhsT=pt_sb, rhs=ident, start=True, stop=True,
                         is_transpose=True)
        cs_ps = tinyps.tile([NSUB, 1], f32, tag="tiny")
        nc.tensor.matmul(cs_ps, lhsT=pt_sb, rhs=ones128, start=True, stop=True)
        cs_sb = smallsb.tile([NSUB, 1], f32, tag="cs")
        nc.vector.tensor_copy(cs_sb, cs_ps)
        sr_ps = tinyps.tile([NSUB, 1], f32, tag="tiny")
        nc.tensor.matmul(sr_ps, lhsT=ones16, rhs=cs_sb, start=True, stop=True)
        rs_sb = smallsb.tile([NSUB, 1], f32, tag="rs")
        nc.vector.reciprocal(rs_sb, sr_ps)

        # outputs go out on the scalar-engine HWDGE queue so they never
        # block the sync-engine load queue.
        w_sb = wpool.tile([NSUB, PP], f32)
        nc.vector.tensor_scalar_mul(out=w_sb, in0=p16_ps, scalar1=rs_sb)
        nc.scalar.dma_start(
            out=w_h[b, 0:15 * PP].rearrange("(g p) -> g p", g=15),
            in_=w_sb[0:15, :])
        nc.scalar.dma_start(
            out=w_h[b, 15 * PP:T].rearrange("(o p) -> o p", o=1),
            in_=w_sb[15:16, 0:T - 15 * PP])

        ctx_sb = ctxsb.tile([1, D_MEM], f32)
        nc.vector.tensor_scalar_mul(out=ctx_sb, in0=ctx_ps,
                                    scalar1=rs_sb[0:1, :])
        nc.scalar.dma_start(out=ctx_h[b].rearrange("(o n) -> o n", o=1),
                            in_=ctx_sb)


_NC_CACHE = {}


def _get_nc():
    key = (CTX_F32R, PL_F32R)
    if key not in _NC_CACHE:
        _NC_CACHE[key] = _build_nc()
    return _NC_CACHE[key]


def _pad_t(x, tp):
    out = np.zeros(x.shape[:-2] + (tp, x.shape[-1]), np.float32)
    out[..., :x.shape[-2], :] = x
    return out


def _marshal_mem(mem):
    # [B, TP, 512] -> [B, 128, 16*512] with t = 128*g + p
    m = _pad_t(mem, TP).reshape(B, NSUB, PP, D_MEM)
    return np.ascontiguousarray(m.transpose(0, 2, 1, 3).reshape(B, PP, NSUB * D_MEM))


def _marshal_pm(pm):
    # [B, TP, 128] -> [B, 128, TP] (d on partitions)
    p = _pad_t(pm, TP)
    return np.ascontiguousarray(p.transpose(0, 2, 1))


def _make_in_maps(inputs):
    hid = np.ascontiguousarray(
        np.asarray(inputs["attention_hidden_state"], np.float32).reshape(B, D_Q))
    mem = _marshal_mem(np.asarray(inputs["memory"], np.float32))
    pm = _marshal_pm(np.asarray(inputs["processed_memory"], np.float32))
    aw_raw = np.asarray(inputs["attention_weights"], np.float32)
    aw = np.zeros((B, 2, PAD + TP + KW), np.float32)
    aw[:, :, PAD:PAD + T] = aw_raw
    wq = np.ascontiguousarray(np.asarray(inputs["Wq"], np.float32))
    wconv = np.ascontiguousarray(
        np.asarray(inputs["Wconv"], np.float32).reshape(C_LOC, 2 * KW))
    bconv = np.ascontiguousarray(np.asarray(inputs["bconv"], np.float32))
    wloc = np.ascontiguousarray(np.asarray(inputs["Wloc"], np.float32))
    v = np.ascontiguousarray(np.asarray(inputs["v"], np.float32))

    in_maps = []
    for c in range(N_CORES):
        s = slice(BPC * c, BPC * (c + 1))
        in_maps.append({
            "hid": hid[s],
            "mem": mem[s],
            "pm": pm[s],
            "aw": np.ascontiguousarray(aw[s].reshape(BPC * 2, -1)),
            "wq": wq,
            "wconv": wconv,
            "bconv": bconv,
            "wloc": wloc,
            "v": v,
        })
    return in_maps


def run(inputs, trace=False):
    nc = _get_nc()
    in_maps = _make_in_maps(inputs)
    res = run_bass_kernel_spmd(nc, in_maps, core_ids=list(range(N_CORES)),
                               trace=trace)
    ctx = np.concatenate([res.results[c]["ctx_out"] for c in range(N_CORES)], 0)
    w = np.concatenate([res.results[c]["w_out"] for c in range(N_CORES)], 0)
    return (ctx, w), res


def kernel(**inputs):
    (ctx, w), _ = run(inputs, trace=False)
    return ctx, w


if __name__ == "__main__":
    nc = _get_nc()
    print("built ok")


# revision 23
# speedup vs baseline: 1.1320x; 1.0957x over previous
"""Trainium2 Bass kernel: Tacotron-style location-sensitive attention.

reference math (per batch b):
  q      = hidden[b] @ Wq                              [1, 128]
  loc    = conv1d(aw[b], Wconv, pad=15) + bconv        [32, T]
  pl     = loc.T @ Wloc                                [T, 128]
  e[t]   = v . tanh(q + pl[t] + pm[b,t])               [T]
  p      = softmax(e)                                  [T]
  ctx    = p @ mem[b]                                  [512]

Sharding: pure data parallel, 8 batches per core on 8 cores.

Layout (per core, 8 batches). T is host-padded 2000->2048 so every on-chip
tile uses the full 128 partitions (125-partition DMAs measure 2.3x slower).
t factors as t = 128*g + p (group g in [0,16), partition p in [0,128)):
  - conv + location projection collapse into ONE matmul per 512-col chunk:
      pl^T[d, t] = Wcomb[0:62].T @ shifted[62, t]
    with shifted[31*ci+k, t] = aw[ci, t+k-15] built by an overlapped-window
    SBUF->SBUF DMA, and Wcomb = Wconv_flat.T @ Wloc computed on device once.
    The bconv term (bconv @ Wloc) and the query q fold into the tanh bias
    (d on partitions).
  - pm tiles are PE-transposed and accumulated into the same PSUM tile as
    the pl matmul (pl first with start=True, transposes start=False).
  - e^T columns [128,1] via PE (lhsT = tanh tile, rhs = v), exp on ACT,
    the 48 padded t slots are zeroed in p, context = 16 accumulating
    matmuls streaming mem tiles [128, 512].
  - softmax norm: column sums via PE + ones, replicated total via a
    ones[16,16] matmul, reciprocal on DVE, scale + DMA out on the scalar
    (ACT) HWDGE queue so stores never block the sync-load queue.
"""

import os
import sys

sys.path.insert(0, "/opt/trn_rl_repo")

import numpy as np
from contextlib import ExitStack

import concourse.bass as bass
import concourse.bacc as bacc
import concourse.tile as tile
from concourse import mybir
from concourse.masks import make_identity
from concourse.bass_utils import run_bass_kernel_spmd

B, T, D_MEM, D_Q, D_ATT, C_LOC, KW = 64, 2000, 512, 1024, 128, 32, 31
PAD = KW // 2  # 15
TP = 2048  # padded T
N_CORES = 8
BPC = B // N_CORES  # 8 batches per core
NCHUNK = 4
TCH = TP // NCHUNK  # 512
NSUB = 16
PP = 128  # partitions; t = 128*g + p
KO = D_Q // 128

f32 = mybir.dt.float32
f32r = mybir.dt.float32r

CTX_F32R = os.environ.get("KERNEL_CTX_F32R", "1") == "1"
PL_F32R = os.environ.get("KERNEL_PL_F32R", "1") == "1"


def _build_nc():
    nc = bacc.Bacc("TRN2", target_bir_lowering=False)

    hid_h = nc.dram_tensor("hid", [BPC, D_Q], f32, kind="ExternalInput")
    mem_h = nc.dram_tensor("mem", [BPC, PP, NSUB * D_MEM], f32,
                           kind="ExternalInput")
    pm_h = nc.dram_tensor("pm", [BPC, D_ATT, TP], f32, kind="ExternalInput")
    aw_h = nc.dram_tensor("aw", [BPC * 2, PAD + TP + KW], f32,
                          kind="ExternalInput")
    wq_h = nc.dram_tensor("wq", [128, KO * D_ATT], f32, kind="ExternalInput")
    wconv_h = nc.dram_tensor("wconv", [C_LOC, 2 * KW], f32, kind="ExternalInput")
    bconv_h = nc.dram_tensor("bconv", [C_LOC], f32, kind="ExternalInput")
    wloc_h = nc.dram_tensor("wloc", [C_LOC, D_ATT], f32, kind="ExternalInput")
    v_h = nc.dram_tensor("v", [D_ATT], f32, kind="ExternalInput")

    ctx_h = nc.dram_tensor("ctx_out", [BPC, D_MEM], f32, kind="ExternalOutput")
    w_h = nc.dram_tensor("w_out", [BPC, T], f32, kind="ExternalOutput")

    with tile.TileContext(nc) as tc, ExitStack() as ctx:
        _body(tc, ctx, hid_h, mem_h, pm_h, aw_h, wq_h, wconv_h, bconv_h,
              wloc_h, v_h, ctx_h, w_h)
    nc.compile()
    return nc


def _body(tc, ctx, hid_h, mem_h, pm_h, aw_h, wq_h, wconv_h, bconv_h, wloc_h,
          v_h, ctx_h, w_h):
    nc = tc.nc
    AF = mybir.ActivationFunctionType
    sh_dt = f32r if PL_F32R else f32
    mem_dt = f32r if CTX_F32R else f32

    singles = ctx.enter_context(tc.tile_pool(name="singles", bufs=1))

    ident = singles.tile([128, 128], f32)
    make_identity(nc, ident)

    # --- one-time weight staging -----------------------------------------
    wcat = singles.tile([C_LOC, 62], f32)
    nc.gpsimd.dma_start(out=wcat, in_=wconv_h[:, :])
    bconv_sb = singles.tile([C_LOC, 1], f32)
    nc.gpsimd.dma_start(out=bconv_sb,
                        in_=bconv_h[:].rearrange("(c o) -> c o", o=1))
    wloc_sb = singles.tile([C_LOC, D_ATT], f32)
    nc.gpsimd.dma_start(out=wloc_sb, in_=wloc_h[:, :])
    v_sb = singles.tile([D_ATT, 1], f32)
    nc.gpsimd.dma_start(out=v_sb, in_=v_h[:].rearrange("(d o) -> d o", o=1))
    wq_sb = singles.tile([128, KO * D_ATT], f32)
    nc.sync.dma_start(out=wq_sb, in_=wq_h[:, :])
    hid_sb = singles.tile([BPC, D_Q], f32)
    nc.sync.dma_start(out=hid_sb, in_=hid_h[:, :])

    ones128 = singles.tile([PP, 1], f32)
    nc.vector.memset(ones128, 1.0)
    # pad mask: 1.0 where t = 128*col + p < 2000, else 0.0
    padmask = singles.tile([PP, NSUB], f32)
    nc.gpsimd.memset(padmask, 1.0)
    nc.gpsimd.affine_select(
        out=padmask, in_=padmask, compare_op=mybir.AluOpType.is_ge,
        fill=0.0, base=T - 1, channel_multiplier=-1,
        pattern=[[-PP, NSUB]])
    ones16 = singles.tile([NSUB, NSUB], f32)
    nc.vector.memset(ones16, 1.0)

    wcomb62 = singles.tile([62, D_ATT], sh_dt)
    qtb = singles.tile([D_ATT, BPC], f32)
    with tc.tile_pool(name="setup_ps", bufs=1, space="PSUM") as sps, \
         tc.tile_pool(name="setup_sb", bufs=2) as ssb:
        # Wcomb = wcat.T @ wloc -> [62, 128]; bconv folds into the tanh
        # bias as Wloc.T @ bconv.
        wcomb_ps = sps.tile([62, D_ATT], f32, tag="wc")
        nc.tensor.matmul(wcomb_ps, lhsT=wcat, rhs=wloc_sb, start=True,
                         stop=True)
        nc.vector.tensor_copy(wcomb62, wcomb_ps)
        bias_ps = sps.tile([D_ATT, 1], f32, tag="bias")
        nc.tensor.matmul(bias_ps, lhsT=wloc_sb, rhs=bconv_sb, start=True,
                         stop=True)
        biaspl = ssb.tile([D_ATT, 1], f32, tag="biaspl")
        nc.vector.tensor_copy(biaspl, bias_ps)

        # hidT [1024, 8] via PE transposes, then qT = Wq.T @ hid.T  [128, 8]
        hidT = ssb.tile([128, KO * BPC], f32, tag="hidT")
        for ko in range(KO):
            ht_ps = sps.tile([128, BPC], f32, tag="ht")
            nc.tensor.transpose(ht_ps, hid_sb[:, 128 * ko:128 * (ko + 1)],
                                ident[0:BPC, 0:BPC])
            nc.vector.tensor_copy(hidT[:, BPC * ko:BPC * (ko + 1)], ht_ps)
        qt_ps = sps.tile([D_ATT, BPC], f32, tag="qt")
        for ko in range(KO):
            nc.tensor.matmul(qt_ps, lhsT=wq_sb[:, 128 * ko:128 * (ko + 1)],
                             rhs=hidT[:, BPC * ko:BPC * (ko + 1)],
                             start=(ko == 0), stop=(ko == KO - 1))
        nc.vector.tensor_scalar(out=qtb, in0=qt_ps, scalar1=biaspl,
                                scalar2=None, op0=mybir.AluOpType.add)

    # --- main pools -------------------------------------------------------
    shpool = ctx.enter_context(tc.tile_pool(name="sh", bufs=3))
    mempool = ctx.enter_context(tc.tile_pool(name="mem", bufs=2))
    pmpool = ctx.enter_context(tc.tile_pool(name="pm", bufs=1))
    thpool = ctx.enter_context(tc.tile_pool(name="th", bufs=4))
    ptpool = ctx.enter_context(tc.tile_pool(name="pt", bufs=3))
    wpool = ctx.enter_context(tc.tile_pool(name="wout", bufs=3))
    ctxsb = ctx.enter_context(tc.tile_pool(name="ctxsb", bufs=3))
    smallsb = ctx.enter_context(tc.tile_pool(name="smallsb", bufs=4))

    p12pool = ctx.enter_context(tc.tile_pool(name="p12", bufs=2, space="PSUM"))
    etpool = ctx.enter_context(tc.tile_pool(name="et", bufs=2, space="PSUM"))
    ctxpool = ctx.enter_context(tc.tile_pool(name="ctxp", bufs=2, space="PSUM"))
    tinyps = ctx.enter_context(tc.tile_pool(name="tinyps", bufs=1, space="PSUM"))

    # ALL of processed_memory is loaded upfront on the scalar-engine HWDGE
    # queue (it fits in SBUF at 64KB/partition), so the per-batch critical
    # path only waits on its own mem tile from the sync queue.
    # processed_memory arrives host-transposed [d, t]; whole-core copy fits
    # in SBUF (64KB/partition) and loads as contiguous 8KB runs per
    # partition on the scalar HWDGE queue.
    pm_all = pmpool.tile([D_ATT, BPC * TP], f32)
    for h in range(BPC):
        nc.scalar.dma_start(out=pm_all[:, h * TP:(h + 1) * TP],
                            in_=pm_h[h])

    for b in range(BPC):
        mem1 = mempool.tile([PP, NSUB * D_MEM], mem_dt)
        src = mem_h[b]
        if CTX_F32R:
            src = src.bitcast(f32r)
        nc.sync.dma_start(out=mem1, in_=src)

        def mem_g(g):
            return mem1[:, g * D_MEM:(g + 1) * D_MEM]

        # shifted windows straight from (host-padded) DRAM:
        # sh[31*ci + k, t] = aw[b, ci, t + k - 15], one overlapped-window DMA
        sh = shpool.tile([62, TP], sh_dt)
        rows = aw_h[2 * b:2 * b + 2, 0:1]
        if PL_F32R:
            rows = rows.bitcast(f32r)
        srcap = bass.AP(tensor=rows.tensor, offset=rows.offset,
                        ap=[list(rows.ap[0]), [1, KW], [1, TP]])
        nc.gpsimd.dma_start(out=sh, in_=srcap)

        eT_ps = etpool.tile([PP, NSUB], f32)
        for c in range(NCHUNK):
            p12 = p12pool.tile([D_ATT, TCH], f32)
            nc.tensor.matmul(p12, lhsT=wcomb62,
                             rhs=sh[:, TCH * c:TCH * (c + 1)],
                             start=True, stop=True)
            t1 = thpool.tile([D_ATT, TCH], f32, tag="t1")
            nc.vector.tensor_add(
                t1, p12, pm_all[:, b * TP + TCH * c:b * TP + TCH * (c + 1)])
            th = thpool.tile([D_ATT, TCH], f32, tag="th")
            nc.scalar.activation(out=th, in_=t1, func=AF.Tanh,
                                 bias=qtb[:, b:b + 1], scale=1.0)
            for j in range(4):
                g = 4 * c + j
                nc.tensor.matmul(eT_ps[:, g:g + 1],
                                 lhsT=th[:, PP * j:PP * (j + 1)],
                                 rhs=v_sb, start=(g == 0), stop=(g == 15),
                                 skip_group_check=True)

        # exp, with the 48 padded t slots (t >= 2000) masked to zero
        pt_raw = ptpool.tile([PP, NSUB], f32, tag="ptraw")
        nc.scalar.activation(out=pt_raw, in_=eT_ps, func=AF.Exp, scale=1.0)
        pt_sb = ptpool.tile([PP, NSUB], f32, tag="pt")
        nc.vector.tensor_mul(pt_sb, pt_raw, padmask)
        if CTX_F32R:
            pt_mm = ptpool.tile([PP, NSUB], f32r, tag="ptr")
            nc.vector.tensor_copy(pt_mm, pt_sb)
        else:
            pt_mm = pt_sb

        ctx_ps = ctxpool.tile([1, D_MEM], f32)
        for g in range(NSUB):
            nc.tensor.matmul(ctx_ps, lhsT=pt_mm[:, g:g + 1], rhs=mem_g(g),
                             start=(g == 0), stop=(g == NSUB - 1),
                             skip_group_check=True)

        # softmax normalization
        p16_ps = tinyps.tile([NSUB, PP], f32, tag="p16")
        nc.tensor.matmul(p16_ps, lhsT=pt_sb, rhs=ident, start=True, stop=True,
                         is_transpose=True)
        cs_ps = tinyps.tile([NSUB, 1], f32, tag="tiny")
        nc.tensor.matmul(cs_ps, lhsT=pt_sb, rhs=ones128, start=True, stop=True)
        cs_sb = smallsb.tile([NSUB, 1], f32, tag="cs")
        nc.vector.tensor_copy(cs_sb, cs_ps)
        sr_ps = tinyps.tile([NSUB, 1], f32, tag="tiny")
        nc.tensor.matmul(sr_ps, lhsT=ones16, rhs=cs_sb, start=True, stop=True)
        rs_sb = smallsb.tile([NSUB, 1], f32, tag="rs")
        nc.vector.reciprocal(rs_sb, sr_ps)

        # outputs go out on the scalar-engine HWDGE queue so they never
        # block the sync-engine load queue.
        w_sb = wpool.tile([NSUB, PP], f32)
        nc.vector.tensor_scalar_mul(out=w_sb, in0=p16_ps, scalar1=rs_sb)
        nc.scalar.dma_start(
            out=w_h[b, 0:15 * PP].rearrange("(g p) -> g p", g=15),
            in_=w_sb[0:15, :])
        nc.scalar.dma_start(
            out=w_h[b, 15 * PP:T].rearrange("(o p) -> o p", o=1),
            in_=w_sb[15:16, 0:T - 15 * PP])

        ctx_sb = ctxsb.tile([1, D_MEM], f32)
        nc.vector.tensor_scalar_mul(out=ctx_sb, in0=ctx_ps,
                                    scalar1=rs_sb[0:1, :])
        nc.scalar.dma_start(out=ctx_h[b].rearrange("(o n) -> o n", o=1),
                            in_=ctx_sb)


_NC_CACHE = {}


def _get_nc():
    key = (CTX_F32R, PL_F32R)
    if key not in _NC_CACHE:
        _NC_CACHE[key] = _build_nc()
    return _NC_CACHE[key]


def _pad_t(x, tp):
    out = np.zeros(x.shape[:-2] + (tp, x.shape[-1]), np.float32)
    out[..., :x.shape[-2], :] = x
    return out


def _marshal_mem(mem):
    # [B, TP, 512] -> [B, 128, 16*512] with t = 128*g + p
    m = _pad_t(mem, TP).reshape(B, NSUB, PP, D_MEM)
    return np.ascontiguousarray(m.transpose(0, 2, 1, 3).reshape(B, PP, NSUB * D_MEM))


def _marshal_pm(pm):
    # [B, TP, 128] -> [B, 128, TP] (d on partitions)
    p = _pad_t(pm, TP)
    return np.ascontiguousarray(p.transpose(0, 2, 1))


def _make_in_maps(inputs):
    hid = np.ascontiguousarray(
        np.asarray(inputs["attention_hidden_state"], np.float32).reshape(B, D_Q))
    mem = _marshal_mem(np.asarray(inputs["memory"], np.float32))
    pm = _marshal_pm(np.asarray(inputs["processed_memory"], np.float32))
    aw_raw = np.asarray(inputs["attention_weights"], np.float32)
    aw = np.zeros((B, 2, PAD + TP + KW), np.float32)
    aw[:, :, PAD:PAD + T] = aw_raw
    wq = np.ascontiguousarray(
        np.asarray(inputs["Wq"], np.float32).reshape(KO, 128, D_ATT)
        .transpose(1, 0, 2).reshape(128, KO * D_ATT))
    wconv = np.ascontiguousarray(
        np.asarray(inputs["Wconv"], np.float32).reshape(C_LOC, 2 * KW))
    bconv = np.ascontiguousarray(np.asarray(inputs["bconv"], np.float32))
    wloc = np.ascontiguousarray(np.asarray(inputs["Wloc"], np.float32))
    v = np.ascontiguousarray(np.asarray(inputs["v"], np.float32))

    in_maps = []
    for c in range(N_CORES):
        s = slice(BPC * c, BPC * (c + 1))
        in_maps.append({
            "hid": hid[s],
            "mem": mem[s],
            "pm": pm[s],
            "aw": np.ascontiguousarray(aw[s].reshape(BPC * 2, -1)),
            "wq": wq,
            "wconv": wconv,
            "bconv": bconv,
            "wloc": wloc,
            "v": v,
        })
    return in_maps


def run(inputs, trace=False):
    nc = _get_nc()
    in_maps = _make_in_maps(inputs)
    res = run_bass_kernel_spmd(nc, in_maps, core_ids=list(range(N_CORES)),
                               trace=trace)
    ctx = np.concatenate([res.results[c]["ctx_out"] for c in range(N_CORES)], 0)
    w = np.concatenate([res.results[c]["w_out"] for c in range(N_CORES)], 0)
    return (ctx, w), res


def kernel(**inputs):
    (ctx, w), _ = run(inputs, trace=False)
    return ctx, w


if __name__ == "__main__":
    nc = _get_nc()
    print("built ok")


# revision 24
# speedup vs baseline: 1.2139x; 1.0724x over previous
"""Trainium2 Bass kernel: Tacotron-style location-sensitive attention.

reference math (per batch b):
  q      = hidden[b] @ Wq                              [1, 128]
  loc    = conv1d(aw[b], Wconv, pad=15) + bconv        [32, T]
  pl     = loc.T @ Wloc                                [T, 128]
  e[t]   = v . tanh(q + pl[t] + pm[b,t])               [T]
  p      = softmax(e)                                  [T]
  ctx    = p @ mem[b]                                  [512]

Sharding: pure data parallel, 8 batches per core on 8 cores.

Layout (per core, 8 batches). T is host-padded 2000->2048 so every on-chip
tile uses the full 128 partitions (125-partition DMAs measure 2.3x slower).
t factors as t = 128*g + p (group g in [0,16), partition p in [0,128)):
  - conv + location projection collapse into ONE matmul per 512-col chunk:
      pl^T[d, t] = Wcomb[0:62].T @ shifted[62, t]
    with shifted[31*ci+k, t] = aw[ci, t+k-15] built by an overlapped-window
    SBUF->SBUF DMA, and Wcomb = Wconv_flat.T @ Wloc computed on device once.
    The bconv term (bconv @ Wloc) and the query q fold into the tanh bias
    (d on partitions).
  - pm tiles are PE-transposed and accumulated into the same PSUM tile as
    the pl matmul (pl first with start=True, transposes start=False).
  - e^T columns [128,1] via PE (lhsT = tanh tile, rhs = v), exp on ACT,
    the 48 padded t slots are zeroed in p, context = 16 accumulating
    matmuls streaming mem tiles [128, 512].
  - softmax norm: column sums via PE + ones, replicated total via a
    ones[16,16] matmul, reciprocal on DVE, scale + DMA out on the scalar
    (ACT) HWDGE queue so stores never block the sync-load queue.
"""

import os
import sys

sys.path.insert(0, "/opt/trn_rl_repo")

import numpy as np
from contextlib import ExitStack

import concourse.bass as bass
import concourse.bacc as bacc
import concourse.tile as tile
from concourse import mybir
from concourse.masks import make_identity
from concourse.bass_utils import run_bass_kernel_spmd

B, T, D_MEM, D_Q, D_ATT, C_LOC, KW = 64, 2000, 512, 1024, 128, 32, 31
PAD = KW // 2  # 15
TP = 2048  # padded T
N_CORES = 8
BPC = B // N_CORES  # 8 batches per core
NCHUNK = 4
TCH = TP // NCHUNK  # 512
NSUB = 16
PP = 128  # partitions; t = 128*g + p
KO = D_Q // 128

f32 = mybir.dt.float32
f32r = mybir.dt.float32r

CTX_F32R = os.environ.get("KERNEL_CTX_F32R", "1") == "1"
PL_F32R = os.environ.get("KERNEL_PL_F32R", "1") == "1"


def _build_nc():
    nc = bacc.Bacc("TRN2", target_bir_lowering=False)

    hid_h = nc.dram_tensor("hid", [BPC, D_Q], f32, kind="ExternalInput")
    mem_h = nc.dram_tensor("mem", [BPC, PP, NSUB * D_MEM], f32,
                           kind="ExternalInput")
    pm_h = nc.dram_tensor("pm", [BPC, D_ATT, TP], f32, kind="ExternalInput")
    aw_h = nc.dram_tensor("aw", [BPC * 2, PAD + TP + KW], f32,
                          kind="ExternalInput")
    wq_h = nc.dram_tensor("wq", [128, KO * D_ATT], f32, kind="ExternalInput")
    wconv_h = nc.dram_tensor("wconv", [C_LOC, 2 * KW], f32, kind="ExternalInput")
    bconv_h = nc.dram_tensor("bconv", [C_LOC], f32, kind="ExternalInput")
    wloc_h = nc.dram_tensor("wloc", [C_LOC, D_ATT], f32, kind="ExternalInput")
    v_h = nc.dram_tensor("v", [D_ATT], f32, kind="ExternalInput")

    ctx_h = nc.dram_tensor("ctx_out", [BPC, D_MEM], f32, kind="ExternalOutput")
    w_h = nc.dram_tensor("w_out", [BPC, T], f32, kind="ExternalOutput")

    with tile.TileContext(nc) as tc, ExitStack() as ctx:
        _body(tc, ctx, hid_h, mem_h, pm_h, aw_h, wq_h, wconv_h, bconv_h,
              wloc_h, v_h, ctx_h, w_h)
    nc.compile()
    return nc


def _body(tc, ctx, hid_h, mem_h, pm_h, aw_h, wq_h, wconv_h, bconv_h, wloc_h,
          v_h, ctx_h, w_h):
    nc = tc.nc
    AF = mybir.ActivationFunctionType
    sh_dt = f32r if PL_F32R else f32
    mem_dt = f32r if CTX_F32R else f32

    singles = ctx.enter_context(tc.tile_pool(name="singles", bufs=1))

    ident = singles.tile([128, 128], f32)
    make_identity(nc, ident)

    # --- one-time weight staging -----------------------------------------
    wcat = singles.tile([C_LOC, 62], f32)
    nc.gpsimd.dma_start(out=wcat, in_=wconv_h[:, :])
    bconv_sb = singles.tile([C_LOC, 1], f32)
    nc.gpsimd.dma_start(out=bconv_sb,
                        in_=bconv_h[:].rearrange("(c o) -> c o", o=1))
    wloc_sb = singles.tile([C_LOC, D_ATT], f32)
    nc.gpsimd.dma_start(out=wloc_sb, in_=wloc_h[:, :])
    v_sb = singles.tile([D_ATT, 1], f32)
    nc.gpsimd.dma_start(out=v_sb, in_=v_h[:].rearrange("(d o) -> d o", o=1))
    wq_sb = singles.tile([128, KO * D_ATT], f32)
    nc.sync.dma_start(out=wq_sb, in_=wq_h[:, :])
    hid_sb = singles.tile([BPC, D_Q], f32)
    nc.sync.dma_start(out=hid_sb, in_=hid_h[:, :])

    ones128 = singles.tile([PP, 1], f32)
    nc.vector.memset(ones128, 1.0)
    # pad mask: 1.0 where t = 128*col + p < 2000, else 0.0
    padmask = singles.tile([PP, NSUB], f32)
    nc.gpsimd.memset(padmask, 1.0)
    nc.gpsimd.affine_select(
        out=padmask, in_=padmask, compare_op=mybir.AluOpType.is_ge,
        fill=0.0, base=T - 1, channel_multiplier=-1,
        pattern=[[-PP, NSUB]])
    ones16 = singles.tile([NSUB, NSUB], f32)
    nc.vector.memset(ones16, 1.0)

    wcomb62 = singles.tile([62, D_ATT], sh_dt)
    qtb = singles.tile([D_ATT, BPC], f32)
    with tc.tile_pool(name="setup_ps", bufs=1, space="PSUM") as sps, \
         tc.tile_pool(name="setup_sb", bufs=2) as ssb:
        # Wcomb = wcat.T @ wloc -> [62, 128]; bconv folds into the tanh
        # bias as Wloc.T @ bconv.
        wcomb_ps = sps.tile([62, D_ATT], f32, tag="wc")
        nc.tensor.matmul(wcomb_ps, lhsT=wcat, rhs=wloc_sb, start=True,
                         stop=True)
        nc.vector.tensor_copy(wcomb62, wcomb_ps)
        bias_ps = sps.tile([D_ATT, 1], f32, tag="bias")
        nc.tensor.matmul(bias_ps, lhsT=wloc_sb, rhs=bconv_sb, start=True,
                         stop=True)
        biaspl = ssb.tile([D_ATT, 1], f32, tag="biaspl")
        nc.vector.tensor_copy(biaspl, bias_ps)

        # hidT [1024, 8] via PE transposes, then qT = Wq.T @ hid.T  [128, 8]
        hidT = ssb.tile([128, KO * BPC], f32, tag="hidT")
        for ko in range(KO):
            ht_ps = sps.tile([128, BPC], f32, tag="ht")
            nc.tensor.transpose(ht_ps, hid_sb[:, 128 * ko:128 * (ko + 1)],
                                ident[0:BPC, 0:BPC])
            nc.vector.tensor_copy(hidT[:, BPC * ko:BPC * (ko + 1)], ht_ps)
        qt_ps = sps.tile([D_ATT, BPC], f32, tag="qt")
        for ko in range(KO):
            nc.tensor.matmul(qt_ps, lhsT=wq_sb[:, 128 * ko:128 * (ko + 1)],
                             rhs=hidT[:, BPC * ko:BPC * (ko + 1)],
                             start=(ko == 0), stop=(ko == KO - 1))
        nc.vector.tensor_scalar(out=qtb, in0=qt_ps, scalar1=biaspl,
                                scalar2=None, op0=mybir.AluOpType.add)

    # --- main pools -------------------------------------------------------
    shpool = ctx.enter_context(tc.tile_pool(name="sh", bufs=3))
    mempool = ctx.enter_context(tc.tile_pool(name="mem", bufs=2))
    pmpool = ctx.enter_context(tc.tile_pool(name="pm", bufs=1))
    thpool = ctx.enter_context(tc.tile_pool(name="th", bufs=4))
    ptpool = ctx.enter_context(tc.tile_pool(name="pt", bufs=3))
    wpool = ctx.enter_context(tc.tile_pool(name="wout", bufs=3))
    ctxsb = ctx.enter_context(tc.tile_pool(name="ctxsb", bufs=3))
    smallsb = ctx.enter_context(tc.tile_pool(name="smallsb", bufs=4))

    p12pool = ctx.enter_context(tc.tile_pool(name="p12", bufs=2, space="PSUM"))
    etpool = ctx.enter_context(tc.tile_pool(name="et", bufs=2, space="PSUM"))
    ctxpool = ctx.enter_context(tc.tile_pool(name="ctxp", bufs=2, space="PSUM"))
    tinyps = ctx.enter_context(tc.tile_pool(name="tinyps", bufs=1, space="PSUM"))

    # ALL of processed_memory is loaded upfront on the scalar-engine HWDGE
    # queue (it fits in SBUF at 64KB/partition), so the per-batch critical
    # path only waits on its own mem tile from the sync queue.
    # processed_memory arrives host-transposed [d, t]; whole-core copy fits
    # in SBUF (64KB/partition) and loads as contiguous 8KB runs per
    # partition on the scalar HWDGE queue.
    pm_all = pmpool.tile([D_ATT, BPC * TP], f32)
    for h in range(BPC):
        nc.scalar.dma_start(out=pm_all[:, h * TP:(h + 1) * TP],
                            in_=pm_h[h])

    for b in range(BPC):
        mem1 = mempool.tile([PP, NSUB * D_MEM], mem_dt)
        src = mem_h[b]
        if CTX_F32R:
            src = src.bitcast(f32r)
        nc.sync.dma_start(out=mem1, in_=src)

        def mem_g(g):
            return mem1[:, g * D_MEM:(g + 1) * D_MEM]

        # shifted windows straight from (host-padded) DRAM:
        # sh[31*ci + k, t] = aw[b, ci, t + k - 15], one overlapped-window DMA
        sh = shpool.tile([62, TP], sh_dt)
        rows = aw_h[2 * b:2 * b + 2, 0:1]
        if PL_F32R:
            rows = rows.bitcast(f32r)
        srcap = bass.AP(tensor=rows.tensor, offset=rows.offset,
                        ap=[list(rows.ap[0]), [1, KW], [1, TP]])
        nc.gpsimd.dma_start(out=sh, in_=srcap)

        eT_ps = etpool.tile([PP, NSUB], f32)
        pt_sb = ptpool.tile([PP, NSUB], f32, tag="pt")
        if CTX_F32R:
            pt_mm = ptpool.tile([PP, NSUB], f32r, tag="ptr")
        else:
            pt_mm = pt_sb
        ctx_ps = ctxpool.tile([1, D_MEM], f32)
        for c in range(NCHUNK):
            p12 = p12pool.tile([D_ATT, TCH], f32)
            nc.tensor.matmul(p12, lhsT=wcomb62,
                             rhs=sh[:, TCH * c:TCH * (c + 1)],
                             start=True, stop=True)
            t1 = thpool.tile([D_ATT, TCH], f32, tag="t1")
            nc.vector.tensor_add(
                t1, p12, pm_all[:, b * TP + TCH * c:b * TP + TCH * (c + 1)])
            th = thpool.tile([D_ATT, TCH], f32, tag="th")
            nc.scalar.activation(out=th, in_=t1, func=AF.Tanh,
                                 bias=qtb[:, b:b + 1], scale=1.0)
            for j in range(4):
                g = 4 * c + j
                nc.tensor.matmul(eT_ps[:, g:g + 1],
                                 lhsT=th[:, PP * j:PP * (j + 1)],
                                 rhs=v_sb, start=(g == 0), stop=(g == 15),
                                 skip_group_check=True)
            # exp + pad mask + context for this chunk's 4 groups, while the
            # next chunk's tanh is still in flight
            sl = slice(4 * c, 4 * c + 4)
            nc.scalar.activation(out=pt_sb[:, sl], in_=eT_ps[:, sl],
                                 func=AF.Exp, scale=1.0)
            nc.vector.tensor_mul(pt_sb[:, sl], pt_sb[:, sl], padmask[:, sl])
            if CTX_F32R:
                nc.vector.tensor_copy(pt_mm[:, sl], pt_sb[:, sl])
            for j in range(4):
                g = 4 * c + j
                nc.tensor.matmul(ctx_ps, lhsT=pt_mm[:, g:g + 1], rhs=mem_g(g),
                                 start=(g == 0), stop=(g == NSUB - 1),
                                 skip_group_check=True)

        # softmax normalization
        p16_ps = tinyps.tile([NSUB, PP], f32, tag="p16")
        nc.tensor.matmul(p16_ps, lhsT=pt_sb, rhs=ident, start=True, stop=True,
                         is_transpose=True)
        cs_ps = tinyps.tile([NSUB, 1], f32, tag="tiny")
        nc.tensor.matmul(cs_ps, lhsT=pt_sb, rhs=ones128, start=True, stop=True)
        cs_sb = smallsb.tile([NSUB, 1], f32, tag="cs")
        nc.vector.tensor_copy(cs_sb, cs_ps)
        sr_ps = tinyps.tile([NSUB, 1], f32, tag="tiny")
        nc.tensor.matmul(sr_ps, lhsT=ones16, rhs=cs_sb, start=True, stop=True)
        rs_sb = smallsb.tile([NSUB, 1], f32, tag="rs")
        nc.vector.reciprocal(rs_sb, sr_ps)

        # outputs go out on the scalar-engine HWDGE queue so they never
        # block the sync-engine load queue.
        w_sb = wpool.tile([NSUB, PP], f32)
        nc.vector.tensor_scalar_mul(out=w_sb, in0=p16_ps, scalar1=rs_sb)
        nc.scalar.dma_start(
            out=w_h[b, 0:15 * PP].rearrange("(g p) -> g p", g=15),
            in_=w_sb[0:15, :])
        nc.scalar.dma_start(
            out=w_h[b, 15 * PP:T].rearrange("(o p) -> o p", o=1),
            in_=w_sb[15:16, 0:T - 15 * PP])

        ctx_sb = ctxsb.tile([1, D_MEM], f32)
        nc.vector.tensor_scalar_mul(out=ctx_sb, in0=ctx_ps,
                                    scalar1=rs_sb[0:1, :])
        nc.scalar.dma_start(out=ctx_h[b].rearrange("(o n) -> o n", o=1),
                            in_=ctx_sb)


_NC_CACHE = {}


def _get_nc():
    key = (CTX_F32R, PL_F32R)
    if key not in _NC_CACHE:
        _NC_CACHE[key] = _build_nc()
    return _NC_CACHE[key]


def _pad_t(x, tp):
    out = np.zeros(x.shape[:-2] + (tp, x.shape[-1]), np.float32)
    out[..., :x.shape[-2], :] = x
    return out


def _marshal_mem(mem):
    # [B, TP, 512] -> [B, 128, 16*512] with t = 128*g + p
    m = _pad_t(mem, TP).reshape(B, NSUB, PP, D_MEM)
    return np.ascontiguousarray(m.transpose(0, 2, 1, 3).reshape(B, PP, NSUB * D_MEM))


def _marshal_pm(pm):
    # [B, TP, 128] -> [B, 128, TP] (d on partitions)
    p = _pad_t(pm, TP)
    return np.ascontiguousarray(p.transpose(0, 2, 1))


def _make_in_maps(inputs):
    hid = np.ascontiguousarray(
        np.asarray(inputs["attention_hidden_state"], np.float32).reshape(B, D_Q))
    mem = _marshal_mem(np.asarray(inputs["memory"], np.float32))
    pm = _marshal_pm(np.asarray(inputs["processed_memory"], np.float32))
    aw_raw = np.asarray(inputs["attention_weights"], np.float32)
    aw = np.zeros((B, 2, PAD + TP + KW), np.float32)
    aw[:, :, PAD:PAD + T] = aw_raw
    wq = np.ascontiguousarray(
        np.asarray(inputs["Wq"], np.float32).reshape(KO, 128, D_ATT)
        .transpose(1, 0, 2).reshape(128, KO * D_ATT))
    wconv = np.ascontiguousarray(
        np.asarray(inputs["Wconv"], np.float32).reshape(C_LOC, 2 * KW))
    bconv = np.ascontiguousarray(np.asarray(inputs["bconv"], np.float32))
    wloc = np.ascontiguousarray(np.asarray(inputs["Wloc"], np.float32))
    v = np.ascontiguousarray(np.asarray(inputs["v"], np.float32))

    in_maps = []
    for c in range(N_CORES):
        s = slice(BPC * c, BPC * (c + 1))
        in_maps.append({
            "hid": hid[s],
            "mem": mem[s],
            "pm": pm[s],
            "aw": np.ascontiguousarray(aw[s].reshape(BPC * 2, -1)),
            "wq": wq,
            "wconv": wconv,
            "bconv": bconv,
            "wloc": wloc,
            "v": v,
        })
    return in_maps


def run(inputs, trace=False):
    nc = _get_nc()
    in_maps = _make_in_maps(inputs)
    res = run_bass_kernel_spmd(nc, in_maps, core_ids=list(range(N_CORES)),
                               trace=trace)
    ctx = np.concatenate([res.results[c]["ctx_out"] for c in range(N_CORES)], 0)
    w = np.concatenate([res.results[c]["w_out"] for c in range(N_CORES)], 0)
    return (ctx, w), res


def kernel(**inputs):
    (ctx, w), _ = run(inputs, trace=False)
    return ctx, w


if __name__ == "__main__":
    nc = _get_nc()
    print("built ok")
